# revision 38
# baseline (speedup 1.0000x reference)
"""JL-GPT2 attention kernel for 8 TRN2 NeuronCores (Bass/Tile).

Sharding (data + head/tensor parallel, per the sharding hint):
  core c -> batch b = c // 4, head group g = c % 4 (4 heads of the 16).
  W_attn columns are split per head group; S_proj (tiny) is folded into the
  q/k weight slices on the host (associativity change only). W_proj rows are
  split per head group, so each core produces a partial [S, D] output;
  partials are summed per batch while unsharding (row-parallel linear),
  b_proj is added once at the end.

Device algorithm (per core), everything fp32 (matmuls in fp32r):
  1. qT/kT = (Wqkjl chunk).T @ hsT   -> JL-projected q,k feature-major
     [128 = 4 heads x 32, S]
  2. v = hsT_block.T @ Wv            -> natural [S, 4 heads x 64]
     stored with a "ones" column per head (row-sum trick)
  3. per (q-block 512, kv-block 128): scoresT = kT_blk.T @ qT_blk (row-packed
     4 heads on the PE array), exp via ScalarE (scale = 1/8 folded in),
     causal handled by skipping upper blocks + restricting column ranges +
     one triangular mask multiply on the diagonal 128x128 sub-block.
  4. outT[65, q] += v_ext.T @ expT   (row 64 accumulates softmax denominators)
  5. normalize: reciprocal(row 64) -> broadcast across partitions (GpSimd) ->
     multiply; write outT feature-major [128 = 2 heads x 64, 2, S]
  6. partial = outT_chunk.T @ W_proj_chunk  -> [S, D], DMA out.
"""

from contextlib import ExitStack

import numpy as np

B, S, D = 2, 2048, 1024
H, HD, K = 16, 64, 32
HPC = 4  # heads per core
NCORES = 8
DC = D // 128  # 8 contraction chunks
QB = 512  # query block (columns of scoresT)
KVB = 128  # kv block (partitions of scoresT)

TRACE = False
TRACE_KWARGS = {}
LAST_RESULT = None
STAGES = "ABCD"  # dev knob: which stages to emit (timing ablation only)

_cache = {}


def _emit_outT(nc, ups, qb, pair, po2, vext_r, prev, nkv):
    import concourse.mybir as mybir

    r, et, lo = prev
    for hh in range(2):
        h = pair * 2 + hh
        if po2[hh] is None:
            po2[hh] = ups.tile(
                [65, 512], mybir.dt.float32, tag="po", bufs=2, name=f"po{qb}_{h}"
            )
        nc.tensor.matmul(
            po2[hh][:, lo:],
            lhsT=vext_r[:, r, h, :],
            rhs=et[:, hh, lo:],
            start=(r == 0),
            stop=(r == nkv - 1),
        )


def _build(has_qkv_bias):
    import concourse.mybir as mybir
    import concourse.tile as tile
    from concourse import bacc

    f32 = mybir.dt.float32
    f32r = mybir.dt.float32r
    Exp = mybir.ActivationFunctionType.Exp
    Identity = mybir.ActivationFunctionType.Identity

    nc = bacc.Bacc("TRN2", target_bir_lowering=False, debug=False)

    hst_d = nc.dram_tensor("hst", [128, DC, S], f32r, kind="ExternalInput").ap()
    wqk_d = nc.dram_tensor("wqk", [128, DC, 256], f32r, kind="ExternalInput").ap()
    wv_d = nc.dram_tensor("wv", [128, DC, 256], f32r, kind="ExternalInput").ap()
    wp_d = nc.dram_tensor("wp", [128, 2, D], f32r, kind="ExternalInput").ap()
    tri_d = nc.dram_tensor("tri", [128, 128], f32, kind="ExternalInput").ap()
    if has_qkv_bias:
        bqk_d = nc.dram_tensor("bqk", [128, 2], f32, kind="ExternalInput").ap()
        bv_d = nc.dram_tensor("bv", [1, 256], f32r, kind="ExternalInput").ap()
    out_d = nc.dram_tensor("out", [S, D], f32, kind="ExternalOutput").ap()

    with tile.TileContext(nc) as tc, ExitStack() as ctx:
        consts = ctx.enter_context(tc.tile_pool(name="consts", bufs=1))
        big = ctx.enter_context(tc.tile_pool(name="big", bufs=1))
        expp = ctx.enter_context(tc.tile_pool(name="expp", bufs=6))
        smallp = ctx.enter_context(tc.tile_pool(name="smallp", bufs=3))
        projp = ctx.enter_context(tc.tile_pool(name="projp", bufs=3))

        # ---- input DMAs (ordering matters: the cost of DMA is serialized
        # at ~360 GB/s per core, so land the qkv weights before the bulk of
        # hsT and wp/tri last) ----
        hst = big.tile([128, DC, S], f32r, tag="hst")
        wqk = big.tile([128, DC, 256], f32r, tag="wqk")
        wv = big.tile([128, DC, 256], f32r, tag="wv")
        wp = big.tile([128, 2, D], f32r, tag="wp")
        tri = consts.tile([128, 128], f32, tag="tri")
        nc.sync.dma_start(wqk[:], wqk_d[:])
        nc.sync.dma_start(wv[:], wv_d[:])
        for c in range(DC):
            nc.sync.dma_start(hst[:, c], hst_d[:, c])
        nc.sync.dma_start(wp[:], wp_d[:])
        nc.sync.dma_start(tri[:], tri_d[:])
        onesf = consts.tile([128, 1], f32, tag="onesf")
        nc.vector.memset(onesf[:], 1.0)
        ones1 = consts.tile([1, 128], f32r, tag="ones1")
        nc.vector.tensor_copy(ones1[:], onesf[0:1, 0:1].to_broadcast([1, 128]))
        if has_qkv_bias:
            bqk = consts.tile([128, 2], f32, tag="bqk")
            nc.sync.dma_start(bqk[:], bqk_d[:])
            bv = consts.tile([1, 256], f32r, tag="bv")
            nc.sync.dma_start(bv[:], bv_d[:])

        qkT = big.tile([128, 2, S], f32r, tag="qkT")  # [4h x 32, {q,k}, S]
        vext = big.tile([128, S // KVB, HPC * 65], f32r, tag="vext")
        vext_r = vext[:].rearrange("p b (h e) -> p b h e", e=65)
        nc.vector.tensor_copy(  # ones column per head
            vext_r[:, :, :, 64:65],
            onesf[:, None, None, :].to_broadcast([128, S // KVB, HPC, 1]),
        )
        outT = big.tile([128, 2, S], f32r, tag="outT")  # [2h x 64, pair, S]

        ups = ctx.enter_context(tc.tile_pool(name="ups", bufs=8, space="PSUM"))

        # ---- stages A-D, interleaved by dependency ----
        # The PE executes in program order, so qkT / v production is emitted
        # in slices right before the attention q-block that first needs them;
        # everything streams against the serialized HBM DMA of hsT.
        # PSUM budget (8 banks): ab 1 + scores 2x2 + out-accumulators 2 +
        # aux (recip-broadcast / projection) 1.
        tri_bc = tri[:, None, :].to_broadcast([128, 2, 128])

        def ab_qk(t, sb):
            ps = ups.tile([128, QB], f32, tag="ab", bufs=1, name=f"qk{t}_{sb}")
            for c in range(DC):
                nc.tensor.matmul(
                    ps[:],
                    lhsT=wqk[:, c, t * 128 : (t + 1) * 128],
                    rhs=hst[:, c, sb * QB : (sb + 1) * QB],
                    start=(c == 0),
                    stop=(c == DC - 1),
                )
            dst = qkT[:, t, sb * QB : (sb + 1) * QB]
            if has_qkv_bias:
                nc.scalar.activation(dst, ps[:], Identity, bias=bqk[:, t : t + 1])
            else:
                nc.vector.tensor_copy(dst, ps[:])

        def ab_v(sb):
            ps = ups.tile([128, 256], f32, tag="ab", bufs=1, name=f"v{sb}")
            for c in range(DC):
                nc.tensor.matmul(
                    ps[:],
                    lhsT=hst[:, c, sb * KVB : (sb + 1) * KVB],
                    rhs=wv[:, c, :],
                    start=(c == 0),
                    stop=(c == DC - 1 and not has_qkv_bias),
                )
            if has_qkv_bias:
                nc.tensor.matmul(
                    ps[:], lhsT=ones1[0:1, :], rhs=bv[0:1, :], start=False, stop=True
                )
            nc.vector.tensor_copy(
                vext_r[:, sb, :, 0:64], ps[:].rearrange("p (h e) -> p h e", e=64)
            )

        def ab_units_for(nqb):
            if nqb >= S // QB:
                return
            if "A" in STAGES:
                ab_qk(0, nqb)
                ab_qk(1, nqb)
            if "B" in STAGES:
                for sb in range(nqb * 4, nqb * 4 + 4):
                    ab_v(sb)

        for qb in range(S // QB):
            if qb == 0:
                ab_units_for(0)
            if "C" not in STAGES:
                ab_units_for(qb + 1)
                continue
            c0 = qb * QB
            nkv = qb * (QB // KVB) + (QB // KVB)
            for pair in range(2):
                if pair == 1:
                    # produce the next q-block's q/k/v while pair 1 streams:
                    # the PE absorbs these in its slack and the copies land
                    # well before the next q-block starts
                    ab_units_for(qb + 1)
                po2 = [None, None]
                prev = None
                for r in range(nkv):
                    o = r * KVB - c0  # diagonal offset of this kv block
                    lo = max(o, 0)
                    ps = ups.tile(
                        [128, 2 * QB], f32, tag="sc", bufs=2, name=f"s{qb}_{pair}_{r}"
                    )
                    ps_r = ps[:].rearrange("p (g q) -> p g q", q=QB)
                    et = expp.tile(
                        [128, 2, QB], f32r, tag="exp", name=f"e{qb}_{pair}_{r}"
                    )
                    for hh in range(2):
                        h = pair * 2 + hh
                        nc.tensor.matmul(
                            ps_r[:, hh, :],
                            lhsT=qkT[
                                h * 32 : (h + 1) * 32, 1, r * KVB : (r + 1) * KVB
                            ],
                            rhs=qkT[h * 32 : (h + 1) * 32, 0, c0 : c0 + QB],
                            start=True,
                            stop=True,
                            tile_position=(32 * h, 0),
                        )
                    # exp(score / sqrt(HD)); no max-subtraction needed (scores
                    # are O(5) by construction, exp cannot overflow fp32)
                    nc.scalar.activation(
                        et[:, :, lo:], ps_r[:, :, lo:], Exp, scale=1.0 / np.sqrt(HD)
                    )
                    if o >= 0:
                        # mask the triangular 128-wide diagonal band (GpSimd:
                        # keeps the exp->outT chain off the busy DVE queue)
                        nc.gpsimd.tensor_mul(
                            et[:, :, o : o + 128], et[:, :, o : o + 128], tri_bc
                        )
                    if prev is not None:
                        _emit_outT(nc, ups, qb, pair, po2, vext_r, prev, nkv)
                    prev = (r, et, lo)
                _emit_outT(nc, ups, qb, pair, po2, vext_r, prev, nkv)
                # normalize: outT_h = po_h[0:64] / po_h[64] (softmax row-sums).
                # Copy each accumulator to SBUF first so its PSUM slot frees
                # after a single DVE op.
                uos = []
                for hh in range(2):
                    uo = smallp.tile(
                        [65, QB], f32, tag="uo", bufs=4, name=f"uo{qb}_{pair}_{hh}"
                    )
                    nc.vector.tensor_copy(uo[:], po2[hh][:])
                    uos.append(uo)
                for hh in range(2):
                    h = pair * 2 + hh
                    rc = smallp.tile([1, QB], f32r, tag="rc")
                    with nc.allow_low_precision(reason="fp32r denominators"):
                        nc.vector.reciprocal(rc[:], uos[hh][64:65, :])
                    pb = ups.tile(
                        [64, QB], f32, tag="aux", bufs=1, name=f"pb{qb}_{h}"
                    )
                    nc.tensor.matmul(
                        pb[:],
                        lhsT=ones1[0:1, 0:64],
                        rhs=rc[0:1, :],
                        start=True,
                        stop=True,
                    )
                    bc = smallp.tile([64, QB], f32, tag="bc")
                    nc.scalar.copy(bc[:], pb[:])
                    nc.vector.tensor_mul(
                        outT[(h % 2) * 64 : (h % 2) * 64 + 64, h // 2, c0 : c0 + QB],
                        uos[hh][0:64, :],
                        bc[:],
                    )
            # ---- projection for this q-block: partial = outT.T @ W_proj ----
            if "D" in STAGES:
                for sb in range(qb * 4, qb * 4 + 4):
                    for nh in range(2):
                        pp = ups.tile(
                            [128, 512], f32, tag="aux", bufs=1, name=f"pp{sb}_{nh}"
                        )
                        for j in range(2):
                            nc.tensor.matmul(
                                pp[:],
                                lhsT=outT[:, j, sb * 128 : (sb + 1) * 128],
                                rhs=wp[:, j, nh * 512 : (nh + 1) * 512],
                                start=(j == 0),
                                stop=(j == 1),
                            )
                        ot = projp.tile([128, 512], f32, tag="projout")
                        nc.vector.tensor_copy(ot[:], pp[:])
                        nc.sync.dma_start(
                            out_d[sb * 128 : (sb + 1) * 128, nh * 512 : (nh + 1) * 512],
                            ot[:],
                        )

    nc.finalize()
    return nc


def _get_nc(has_qkv_bias):
    key = bool(has_qkv_bias)
    if key not in _cache:
        _cache[key] = _build(key)
    return _cache[key]


def kernel(hidden_states, W_attn, b_attn, S_proj, W_proj, b_proj):
    global LAST_RESULT
    from concourse.bass_utils import run_bass_kernel_spmd

    hs = np.asarray(hidden_states, np.float32)
    W_attn = np.asarray(W_attn, np.float32)
    b_attn = np.asarray(b_attn, np.float32)
    S_proj = np.asarray(S_proj, np.float32)
    W_proj = np.asarray(W_proj, np.float32)
    b_proj = np.asarray(b_proj, np.float32)

    has_bias = bool(np.any(b_attn))
    nc = _get_nc(has_bias)

    SpT = S_proj.T  # [HD, K]
    tri = np.triu(np.ones((128, 128), np.float32))  # valid where q_local >= kv_local
    GW = HPC * HD  # 256 columns per head group

    def sw(a, chunks, width):  # [D-like, width] -> [128, chunks, width] SBUF layout
        return np.ascontiguousarray(a.reshape(chunks, 128, width).transpose(1, 0, 2))

    in_maps = []
    for core in range(NCORES):
        b, g = divmod(core, 4)
        Wq = W_attn[:, 0 * D + g * GW : 0 * D + (g + 1) * GW]
        Wk = W_attn[:, 1 * D + g * GW : 1 * D + (g + 1) * GW]
        Wv = W_attn[:, 2 * D + g * GW : 2 * D + (g + 1) * GW]
        Wq_jl = (Wq.reshape(D, HPC, HD) @ SpT).reshape(D, HPC * K)
        Wk_jl = (Wk.reshape(D, HPC, HD) @ SpT).reshape(D, HPC * K)
        wqk = np.concatenate([Wq_jl, Wk_jl], axis=1)  # [D, 256]
        m = {
            "hst": sw(hs[b].T, DC, S),
            "wqk": sw(wqk, DC, 256),
            "wv": sw(Wv, DC, 256),
            "wp": sw(W_proj[g * GW : (g + 1) * GW, :], 2, D),
            "tri": tri,
        }
        if has_bias:
            bq = b_attn[0 * D + g * GW : 0 * D + (g + 1) * GW]
            bk = b_attn[1 * D + g * GW : 1 * D + (g + 1) * GW]
            bv = b_attn[2 * D + g * GW : 2 * D + (g + 1) * GW]
            bq_jl = (bq.reshape(HPC, HD) @ SpT).reshape(HPC * K)
            bk_jl = (bk.reshape(HPC, HD) @ SpT).reshape(HPC * K)
            m["bqk"] = np.ascontiguousarray(np.stack([bq_jl, bk_jl], axis=1))
            m["bv"] = np.ascontiguousarray(bv.reshape(1, GW))
        in_maps.append(m)

    res = run_bass_kernel_spmd(
        nc,
        in_maps,
        core_ids=list(range(NCORES)),
        trace=TRACE,
        **TRACE_KWARGS,
    )
    LAST_RESULT = res

    out = np.zeros((B, S, D), np.float32)
    for core in range(NCORES):
        b, _ = divmod(core, 4)
        out[b] += res.results[core]["out"]
    out += b_proj
    return out


# revision 39
# speedup vs baseline: 19140.3667x; 19140.3667x over previous
"""JL-GPT2 attention kernel for 8 TRN2 NeuronCores (Bass/Tile).

Sharding (data + head/tensor parallel, per the sharding hint):
  core c -> batch b = c // 4, head group g = c % 4 (4 heads of the 16).
  W_attn columns are split per head group; S_proj (tiny) is folded into the
  q/k weight slices on the host (associativity change only). W_proj rows are
  split per head group, so each core produces a partial [S, D] output;
  partials are summed per batch while unsharding (row-parallel linear),
  b_proj is added once at the end.

Device algorithm (per core), everything fp32 (matmuls in fp32r):
  1. qT/kT = (Wqkjl chunk).T @ hsT   -> JL-projected q,k feature-major
     [128 = 4 heads x 32, S]
  2. v = hsT_block.T @ Wv            -> natural [S, 4 heads x 64]
     stored with a "ones" column per head (row-sum trick)
  3. per (q-block 512, kv-block 128): scoresT = kT_blk.T @ qT_blk (row-packed
     4 heads on the PE array), exp via ScalarE (scale = 1/8 folded in),
     causal handled by skipping upper blocks + restricting column ranges +
     one triangular mask multiply on the diagonal 128x128 sub-block.
  4. outT[65, q] += v_ext.T @ expT   (row 64 accumulates softmax denominators)
  5. normalize: reciprocal(row 64) -> broadcast across 64 partitions via a
     K=1 PE outer product -> multiply; outT feature-major [2 heads x 64, 2, S]
  6. partial = outT_chunk.T @ W_proj_chunk  -> [S, D], DMA out.
"""

from contextlib import ExitStack

import numpy as np

B, S, D = 2, 2048, 1024
H, HD, K = 16, 64, 32
HPC = 4  # heads per core
NCORES = 8
DC = D // 128  # 8 contraction chunks
QB = 512  # query block (columns of scoresT)
KVB = 128  # kv block (partitions of scoresT)

TRACE = False
TRACE_KWARGS = {}
LAST_RESULT = None
STAGES = "ABCD"  # dev knob: which stages to emit (timing ablation only)

_cache = {}


def _emit_outT(nc, ups, qb, pair, po2, vext_r, prev, nkv):
    import concourse.mybir as mybir

    r, et, lo = prev
    for hh in range(2):
        h = pair * 2 + hh
        if po2[hh] is None:
            po2[hh] = ups.tile(
                [65, 512], mybir.dt.float32, tag="po", bufs=2, name=f"po{qb}_{h}"
            )
        nc.tensor.matmul(
            po2[hh][:, lo:],
            lhsT=vext_r[:, r, h, :],
            rhs=et[:, hh, lo:],
            start=(r == 0),
            stop=(r == nkv - 1),
        )


def _build(has_qkv_bias):
    import concourse.mybir as mybir
    import concourse.tile as tile
    from concourse import bacc

    f32 = mybir.dt.float32
    f32r = mybir.dt.float32r
    Exp = mybir.ActivationFunctionType.Exp
    Identity = mybir.ActivationFunctionType.Identity

    nc = bacc.Bacc("TRN2", target_bir_lowering=False, debug=False)

    hst_d = nc.dram_tensor("hst", [128, DC, S], f32r, kind="ExternalInput").ap()
    wqk_d = nc.dram_tensor("wqk", [128, DC, 256], f32r, kind="ExternalInput").ap()
    wv_d = nc.dram_tensor("wv", [128, DC, 256], f32r, kind="ExternalInput").ap()
    wp_d = nc.dram_tensor("wp", [128, 2, D], f32r, kind="ExternalInput").ap()
    tri_d = nc.dram_tensor("tri", [128, 128], f32, kind="ExternalInput").ap()
    if has_qkv_bias:
        bqk_d = nc.dram_tensor("bqk", [128, 2], f32, kind="ExternalInput").ap()
        bv_d = nc.dram_tensor("bv", [1, 256], f32r, kind="ExternalInput").ap()
    out_d = nc.dram_tensor("out", [S, D], f32, kind="ExternalOutput").ap()

    with tile.TileContext(nc) as tc, ExitStack() as ctx:
        consts = ctx.enter_context(tc.tile_pool(name="consts", bufs=1))
        big = ctx.enter_context(tc.tile_pool(name="big", bufs=1))
        expp = ctx.enter_context(tc.tile_pool(name="expp", bufs=6))
        smallp = ctx.enter_context(tc.tile_pool(name="smallp", bufs=3))
        projp = ctx.enter_context(tc.tile_pool(name="projp", bufs=3))

        # ---- input DMAs (ordering matters: the cost of DMA is serialized
        # at ~360 GB/s per core, so land the qkv weights before the bulk of
        # hsT and wp/tri last) ----
        hst = big.tile([128, DC, S], f32r, tag="hst")
        wqk = big.tile([128, DC, 256], f32r, tag="wqk")
        wv = big.tile([128, DC, 256], f32r, tag="wv")
        wp = big.tile([128, 2, D], f32r, tag="wp")
        tri = consts.tile([128, 128], f32, tag="tri")
        nc.sync.dma_start(wqk[:], wqk_d[:])
        nc.sync.dma_start(wv[:], wv_d[:])
        for c in range(DC):
            nc.sync.dma_start(hst[:, c], hst_d[:, c])
        nc.sync.dma_start(wp[:], wp_d[:])
        nc.sync.dma_start(tri[:], tri_d[:])
        onesf = consts.tile([128, 1], f32, tag="onesf")
        nc.vector.memset(onesf[:], 1.0)
        ones1 = consts.tile([1, 128], f32r, tag="ones1")
        nc.vector.tensor_copy(ones1[:], onesf[0:1, 0:1].to_broadcast([1, 128]))
        if has_qkv_bias:
            bqk = consts.tile([128, 2], f32, tag="bqk")
            nc.sync.dma_start(bqk[:], bqk_d[:])
            bv = consts.tile([1, 256], f32r, tag="bv")
            nc.sync.dma_start(bv[:], bv_d[:])

        qkT = big.tile([128, 2, S], f32r, tag="qkT")  # [4h x 32, {q,k}, S]
        vext = big.tile([128, S // KVB, HPC * 65], f32r, tag="vext")
        vext_r = vext[:].rearrange("p b (h e) -> p b h e", e=65)
        nc.vector.tensor_copy(  # ones column per head
            vext_r[:, :, :, 64:65],
            onesf[:, None, None, :].to_broadcast([128, S // KVB, HPC, 1]),
        )
        outT = big.tile([128, 2, S], f32r, tag="outT")  # [2h x 64, pair, S]

        ups = ctx.enter_context(tc.tile_pool(name="ups", bufs=8, space="PSUM"))

        # ---- stages A-D, interleaved by dependency ----
        # The PE executes in program order, so qkT / v production is emitted
        # in slices right before the attention q-block that first needs them;
        # everything streams against the serialized HBM DMA of hsT.
        # PSUM budget (8 banks): ab 1 + scores 2x2 + out-accumulators 2 +
        # aux (recip-broadcast / projection) 1.
        tri_bc = tri[:, None, :].to_broadcast([128, 2, 128])

        def ab_qk(t, sb):
            ps = ups.tile([128, QB], f32, tag="ab", bufs=1, name=f"qk{t}_{sb}")
            for c in range(DC):
                nc.tensor.matmul(
                    ps[:],
                    lhsT=wqk[:, c, t * 128 : (t + 1) * 128],
                    rhs=hst[:, c, sb * QB : (sb + 1) * QB],
                    start=(c == 0),
                    stop=(c == DC - 1),
                )
            dst = qkT[:, t, sb * QB : (sb + 1) * QB]
            if has_qkv_bias:
                nc.scalar.activation(dst, ps[:], Identity, bias=bqk[:, t : t + 1])
            else:
                nc.vector.tensor_copy(dst, ps[:])

        def ab_v(sb):
            ps = ups.tile([128, 256], f32, tag="ab", bufs=1, name=f"v{sb}")
            for c in range(DC):
                nc.tensor.matmul(
                    ps[:],
                    lhsT=hst[:, c, sb * KVB : (sb + 1) * KVB],
                    rhs=wv[:, c, :],
                    start=(c == 0),
                    stop=(c == DC - 1 and not has_qkv_bias),
                )
            if has_qkv_bias:
                nc.tensor.matmul(
                    ps[:], lhsT=ones1[0:1, :], rhs=bv[0:1, :], start=False, stop=True
                )
            nc.vector.tensor_copy(
                vext_r[:, sb, :, 0:64], ps[:].rearrange("p (h e) -> p h e", e=64)
            )

        def ab_units_for(nqb):
            if nqb >= S // QB:
                return
            if "A" in STAGES:
                ab_qk(0, nqb)
                ab_qk(1, nqb)
            if "B" in STAGES:
                for sb in range(nqb * 4, nqb * 4 + 4):
                    ab_v(sb)

        for qb in range(S // QB):
            if qb == 0:
                ab_units_for(0)
            if "C" not in STAGES:
                ab_units_for(qb + 1)
                continue
            c0 = qb * QB
            nkv = qb * (QB // KVB) + (QB // KVB)
            for pair in range(2):
                if pair == 1:
                    # produce the next q-block's q/k/v while pair 1 streams:
                    # the PE absorbs these in its slack and the copies land
                    # well before the next q-block starts
                    ab_units_for(qb + 1)
                po2 = [None, None]
                prev = None
                for r in range(nkv):
                    o = r * KVB - c0  # diagonal offset of this kv block
                    lo = max(o, 0)
                    ps = ups.tile(
                        [128, 2 * QB], f32, tag="sc", bufs=2, name=f"s{qb}_{pair}_{r}"
                    )
                    ps_r = ps[:].rearrange("p (g q) -> p g q", q=QB)
                    et = expp.tile(
                        [128, 2, QB], f32r, tag="exp", name=f"e{qb}_{pair}_{r}"
                    )
                    for hh in range(2):
                        h = pair * 2 + hh
                        nc.tensor.matmul(
                            ps_r[:, hh, :],
                            lhsT=qkT[
                                h * 32 : (h + 1) * 32, 1, r * KVB : (r + 1) * KVB
                            ],
                            rhs=qkT[h * 32 : (h + 1) * 32, 0, c0 : c0 + QB],
                            start=True,
                            stop=True,
                            tile_position=(32 * h, 0),
                        )
                    # exp(score / sqrt(HD)); no max-subtraction needed (scores
                    # are O(5) by construction, exp cannot overflow fp32)
                    nc.scalar.activation(
                        et[:, :, lo:], ps_r[:, :, lo:], Exp, scale=1.0 / np.sqrt(HD)
                    )
                    if o >= 0:
                        # mask the triangular 128-wide diagonal band (GpSimd:
                        # keeps the exp->outT chain off the busy DVE queue)
                        nc.gpsimd.tensor_mul(
                            et[:, :, o : o + 128], et[:, :, o : o + 128], tri_bc
                        )
                    if prev is not None:
                        _emit_outT(nc, ups, qb, pair, po2, vext_r, prev, nkv)
                    prev = (r, et, lo)
                _emit_outT(nc, ups, qb, pair, po2, vext_r, prev, nkv)
                # normalize: outT_h = po_h[0:64] / po_h[64] (softmax row-sums).
                # Copy each accumulator to SBUF first so its PSUM slot frees
                # after a single DVE op.
                uos = []
                for hh in range(2):
                    uo = smallp.tile(
                        [65, QB], f32, tag="uo", bufs=4, name=f"uo{qb}_{pair}_{hh}"
                    )
                    nc.vector.tensor_copy(uo[:], po2[hh][:])
                    uos.append(uo)
                for hh in range(2):
                    h = pair * 2 + hh
                    rc = smallp.tile([1, QB], f32r, tag="rc")
                    with nc.allow_low_precision(reason="fp32r denominators"):
                        nc.vector.reciprocal(rc[:], uos[hh][64:65, :])
                    pb = ups.tile(
                        [64, QB], f32, tag="aux", bufs=1, name=f"pb{qb}_{h}"
                    )
                    nc.tensor.matmul(
                        pb[:],
                        lhsT=ones1[0:1, 0:64],
                        rhs=rc[0:1, :],
                        start=True,
                        stop=True,
                    )
                    bc = smallp.tile([64, QB], f32, tag="bc")
                    nc.scalar.copy(bc[:], pb[:])
                    nc.vector.tensor_mul(
                        outT[(h % 2) * 64 : (h % 2) * 64 + 64, h // 2, c0 : c0 + QB],
                        uos[hh][0:64, :],
                        bc[:],
                    )
            # ---- projection for this q-block: partial = outT.T @ W_proj ----
            if "D" in STAGES:
                for sb in range(qb * 4, qb * 4 + 4):
                    for nh in range(2):
                        pp = ups.tile(
                            [128, 512], f32, tag="aux", bufs=1, name=f"pp{sb}_{nh}"
                        )
                        for j in range(2):
                            nc.tensor.matmul(
                                pp[:],
                                lhsT=outT[:, j, sb * 128 : (sb + 1) * 128],
                                rhs=wp[:, j, nh * 512 : (nh + 1) * 512],
                                start=(j == 0),
                                stop=(j == 1),
                            )
                        ot = projp.tile([128, 512], f32, tag="projout")
                        nc.vector.tensor_copy(ot[:], pp[:])
                        nc.sync.dma_start(
                            out_d[sb * 128 : (sb + 1) * 128, nh * 512 : (nh + 1) * 512],
                            ot[:],
                        )

    nc.finalize()
    return nc


def _get_nc(has_qkv_bias):
    key = bool(has_qkv_bias)
    if key not in _cache:
        _cache[key] = _build(key)
    return _cache[key]


def kernel(hidden_states, W_attn, b_attn, S_proj, W_proj, b_proj):
    global LAST_RESULT
    from concourse.bass_utils import run_bass_kernel_spmd

    hs = np.asarray(hidden_states, np.float32)
    W_attn = np.asarray(W_attn, np.float32)
    b_attn = np.asarray(b_attn, np.float32)
    S_proj = np.asarray(S_proj, np.float32)
    W_proj = np.asarray(W_proj, np.float32)
    b_proj = np.asarray(b_proj, np.float32)

    has_bias = bool(np.any(b_attn))
    nc = _get_nc(has_bias)

    SpT = S_proj.T  # [HD, K]
    tri = np.triu(np.ones((128, 128), np.float32))  # valid where q_local >= kv_local
    GW = HPC * HD  # 256 columns per head group

    def sw(a, chunks, width):  # [D-like, width] -> [128, chunks, width] SBUF layout
        return np.ascontiguousarray(a.reshape(chunks, 128, width).transpose(1, 0, 2))

    in_maps = []
    for core in range(NCORES):
        b, g = divmod(core, 4)
        Wq = W_attn[:, 0 * D + g * GW : 0 * D + (g + 1) * GW]
        Wk = W_attn[:, 1 * D + g * GW : 1 * D + (g + 1) * GW]
        Wv = W_attn[:, 2 * D + g * GW : 2 * D + (g + 1) * GW]
        Wq_jl = (Wq.reshape(D, HPC, HD) @ SpT).reshape(D, HPC * K)
        Wk_jl = (Wk.reshape(D, HPC, HD) @ SpT).reshape(D, HPC * K)
        wqk = np.concatenate([Wq_jl, Wk_jl], axis=1)  # [D, 256]
        m = {
            "hst": sw(hs[b].T, DC, S),
            "wqk": sw(wqk, DC, 256),
            "wv": sw(Wv, DC, 256),
            "wp": sw(W_proj[g * GW : (g + 1) * GW, :], 2, D),
            "tri": tri,
        }
        if has_bias:
            bq = b_attn[0 * D + g * GW : 0 * D + (g + 1) * GW]
            bk = b_attn[1 * D + g * GW : 1 * D + (g + 1) * GW]
            bv = b_attn[2 * D + g * GW : 2 * D + (g + 1) * GW]
            bq_jl = (bq.reshape(HPC, HD) @ SpT).reshape(HPC * K)
            bk_jl = (bk.reshape(HPC, HD) @ SpT).reshape(HPC * K)
            m["bqk"] = np.ascontiguousarray(np.stack([bq_jl, bk_jl], axis=1))
            m["bv"] = np.ascontiguousarray(bv.reshape(1, GW))
        in_maps.append(m)

    res = run_bass_kernel_spmd(
        nc,
        in_maps,
        core_ids=list(range(NCORES)),
        trace=TRACE,
        **TRACE_KWARGS,
    )
    LAST_RESULT = res

    out = np.zeros((B, S, D), np.float32)
    for core in range(NCORES):
        b, _ = divmod(core, 4)
        out[b] += res.results[core]["out"]
    out += b_proj
    return out
